# revision 2
# baseline (speedup 1.0000x reference)
"""KKAN Convolutional Network kernel for 8 Trainium2 NeuronCores.

Data parallel over batch (32 images -> 4 per core). The KAN conv is
reformulated as a pointwise feature expansion followed by a dense 3x3
conv. Key trick: each cubic B-spline basis B_g(x) on the uniform grid
is a fixed 5-tap combination of shifted relu^3 maps,
  B_g(u) = (R_g - 4 R_{g+1} + 6 R_{g+2} - 4 R_{g+3} + R_{g+4}) / 6,
  R_j = relu(u - j)^3,  u = (x + 2.2) * 2.5.
The constant 5-tap combination is folded into the conv weights, so the
per-pixel nonlinear work is just silu(x) plus 12 shifted relu-cubes
(~5 tensor ops total) instead of the full Cox-de-Boor recursion
(~170 map-ops). Feature channels: [silu(x), R_0..R_11] = 13; conv1
contraction = 13*9 = 117 <= 128 (single PE pass per tile).
"""
import numpy as np
import jax
import jax.numpy as jnp
from functools import partial

GRID_SIZE = 5
SPLINE_ORDER = 3
N_CONVS = 16
K = 3
P = K * K
G = GRID_SIZE + SPLINE_ORDER  # 8
N_CORES = 8
B, H, W = 32, 256, 256
NR = G + SPLINE_ORDER + 1  # 12 relu-cube maps
UCLIP = 14.0  # clamp u: all bases identically 0 for u>=11; keeps cancellation sane


@partial(jax.pmap, in_axes=(0, None, None, None), devices=jax.devices()[:N_CORES])
def _run_shard(x, w1, rw, rb):
    # x: (n,1,H,W); w1: (16, 1+NR, 3, 3); rw: (1,16,3,3); rb: (1,)
    xx = x[:, 0]  # (n,H,W)
    sil = jax.nn.silu(xx)
    u = jnp.clip((xx + 2.2) * 2.5, -1.0, UCLIP)
    v = u[:, None] - jnp.arange(NR, dtype=jnp.float32)[None, :, None, None]
    r = jnp.maximum(v, 0.0)
    rc = r * r * r  # (n, 12, H, W)
    feats = jnp.concatenate([sil[:, None], rc], axis=1)  # (n, 13, H, W)
    feat = jax.lax.conv_general_dilated(
        feats, w1, (1, 1), [(1, 1), (1, 1)],
        dimension_numbers=('NCHW', 'OIHW', 'NCHW'))  # (n,16,H,W)
    y = jax.lax.conv_general_dilated(
        feat, rw, (1, 1), [(1, 1), (1, 1)],
        dimension_numbers=('NCHW', 'OIHW', 'NCHW'))
    return y + rb[None, :, None, None]


def _fold_weights(base_w, spline_w, spline_scaler):
    # scaled spline weights (16, 9, 8) -> relu-cube weights (16, 9, 12)
    sw = (np.asarray(spline_w) * np.asarray(spline_scaler)[..., None]).astype(np.float64)
    c5 = np.array([1.0, -4.0, 6.0, -4.0, 1.0], np.float64) / 6.0
    wr = np.zeros((N_CONVS, P, NR), np.float64)
    for g in range(G):
        for k in range(5):
            wr[:, :, g + k] += sw[:, :, g] * c5[k]
    # B-spline argument is u - g with u=(x+2.2)/0.4; relu cubes computed on the
    # same u, so no further scaling needed.
    bw = np.asarray(base_w, np.float64)
    w1 = np.zeros((N_CONVS, 1 + NR, K, K), np.float32)
    for di in range(K):
        for dj in range(K):
            p = di * K + dj
            w1[:, 0, di, dj] = bw[:, p]
            w1[:, 1:, di, dj] = wr[:, p, :]
    return w1


def kernel(x, base_w, spline_w, spline_scaler, restore_w, restore_b):
    x = np.asarray(x, np.float32)
    w1 = _fold_weights(base_w, spline_w, spline_scaler)
    xs = x.reshape(N_CORES, B // N_CORES, 1, H, W)
    y = _run_shard(xs, jnp.asarray(w1), jnp.asarray(restore_w, np.float32),
                   jnp.asarray(restore_b, np.float32))
    return np.asarray(y).reshape(B, 1, H, W)


# revision 3
# speedup vs baseline: 1.2289x; 1.2289x over previous
"""KKAN Convolutional Network kernel for 8 Trainium2 NeuronCores.

Data parallel over batch (32 images -> 4 per core). Two key
reformulations vs the straightforward lowering:

1. B-spline bases via the ReLU^3 identity: each cubic B-spline basis
   on the uniform grid is a fixed 5-tap combination of shifted
   relu(u-j)^3 maps (u = (x+2.2)*2.5). The constant combination is
   folded into the conv weights, so the per-pixel nonlinear work is
   silu(x) + 12 shifted relu-cubes (~5 tensor ops) instead of the
   Cox-de-Boor recursion (~170 map-ops). Feature channels:
   [silu(x), R_0..R_11] = 13.

2. Both 3x3 convs are expressed as 9 accumulated pointwise dots on
   shifted views instead of lax.conv_general_dilated -- the XLA-neuron
   conv lowering inserts pf_transpose kernels and is ~6x slower than
   the tap-sum GEMM form on this shape.
"""
import numpy as np
import jax
import jax.numpy as jnp
from functools import partial

GRID_SIZE = 5
SPLINE_ORDER = 3
N_CONVS = 16
K = 3
P = K * K
G = GRID_SIZE + SPLINE_ORDER  # 8
N_CORES = 8
B, H, W = 32, 256, 256
NR = G + SPLINE_ORDER + 1  # 12 relu-cube maps
UCLIP = 14.0  # all bases are identically 0 for u>=11; clamp keeps cancellation sane


def _feats(x):
    xx = x[:, 0]  # (n,H,W)
    sil = jax.nn.silu(xx)
    u = jnp.clip((xx + 2.2) * 2.5, -1.0, UCLIP)
    v = u[:, None] - jnp.arange(NR, dtype=jnp.float32)[None, :, None, None]
    r = jnp.maximum(v, 0.0)
    return jnp.concatenate([sil[:, None], r * r * r], axis=1)  # (n,13,H,W)


def _conv1(feats, w1):
    n, C, HH, WW = feats.shape
    fp = jnp.pad(feats, ((0, 0), (0, 0), (1, 1), (1, 1)))
    out = None
    for di in range(K):
        for dj in range(K):
            v = fp[:, :, di:di + HH, dj:dj + WW]
            p = jnp.einsum('nchw,oc->nohw', v, w1[:, :, di, dj])
            out = p if out is None else out + p
    return out  # (n,16,H,W)


def _conv2(feat, rw):
    n, C, HH, WW = feat.shape
    fp = jnp.pad(feat, ((0, 0), (0, 0), (1, 1), (1, 1)))
    out = None
    for di in range(K):
        for dj in range(K):
            v = fp[:, :, di:di + HH, dj:dj + WW]
            p = jnp.einsum('nchw,c->nhw', v, rw[0, :, di, dj])
            out = p if out is None else out + p
    return out[:, None]  # (n,1,H,W)


@partial(jax.pmap, in_axes=(0, None, None, None), devices=jax.devices()[:N_CORES])
def _run_shard(x, w1, rw, rb):
    y = _conv2(_conv1(_feats(x), w1), rw)
    return y + rb[None, :, None, None]


def _fold_weights(base_w, spline_w, spline_scaler):
    # scaled spline weights (16, 9, 8) -> relu-cube weights (16, 9, 12)
    sw = (np.asarray(spline_w) * np.asarray(spline_scaler)[..., None]).astype(np.float64)
    c5 = np.array([1.0, -4.0, 6.0, -4.0, 1.0], np.float64) / 6.0
    wr = np.zeros((N_CONVS, P, NR), np.float64)
    for g in range(G):
        for k in range(5):
            wr[:, :, g + k] += sw[:, :, g] * c5[k]
    bw = np.asarray(base_w, np.float64)
    w1 = np.zeros((N_CONVS, 1 + NR, K, K), np.float32)
    for di in range(K):
        for dj in range(K):
            p = di * K + dj
            w1[:, 0, di, dj] = bw[:, p]
            w1[:, 1:, di, dj] = wr[:, p, :]
    return w1


def kernel(x, base_w, spline_w, spline_scaler, restore_w, restore_b):
    x = np.asarray(x, np.float32)
    w1 = _fold_weights(base_w, spline_w, spline_scaler)
    xs = x.reshape(N_CORES, B // N_CORES, 1, H, W)
    y = _run_shard(xs, jnp.asarray(w1), jnp.asarray(restore_w, np.float32),
                   jnp.asarray(restore_b, np.float32))
    return np.asarray(y).reshape(B, 1, H, W)


# revision 4
# speedup vs baseline: 1.2642x; 1.0287x over previous
"""KKAN Convolutional Network kernel for 8 Trainium2 NeuronCores.

Data parallel over batch (32 images -> 4 per core). Two key
reformulations vs the straightforward lowering:

1. B-spline bases via the ReLU^3 identity: each cubic B-spline basis
   on the uniform grid is a fixed 5-tap combination of shifted
   relu(u-j)^3 maps (u = (x+2.2)*2.5). The constant combination is
   folded into the conv weights, so the per-pixel nonlinear work is
   silu(x) + 12 shifted relu-cubes (~5 tensor ops) instead of the
   Cox-de-Boor recursion (~170 map-ops). Feature channels:
   [silu(x), R_0..R_11] = 13.

2. Both 3x3 convs are expressed as 9 accumulated pointwise dots on
   shifted views instead of lax.conv_general_dilated -- the XLA-neuron
   conv lowering inserts pf_transpose kernels and is ~6x slower than
   the tap-sum GEMM form on this shape.
"""
import numpy as np
import jax
import jax.numpy as jnp
from functools import partial

GRID_SIZE = 5
SPLINE_ORDER = 3
N_CONVS = 16
K = 3
P = K * K
G = GRID_SIZE + SPLINE_ORDER  # 8
N_CORES = 8
B, H, W = 32, 256, 256
NR = G + SPLINE_ORDER + 1  # 12 relu-cube maps
UCLIP = 14.0  # all bases are identically 0 for u>=11; clamp keeps cancellation sane


def _feats(x):
    xx = x[:, 0]  # (n,H,W)
    sil = jax.nn.silu(xx)
    u = jnp.clip((xx + 2.2) * 2.5, -1.0, UCLIP)
    v = u[:, None] - jnp.arange(NR, dtype=jnp.float32)[None, :, None, None]
    r = jnp.maximum(v, 0.0)
    return jnp.concatenate([sil[:, None], r * r * r], axis=1)  # (n,13,H,W)


def _conv1(feats, w1):
    # Concat the 3 W-shifts into 39 channels, then 3 fat-K GEMMs over the
    # H-shifts (3x fewer PE column-streams than 9 thin-K tap dots).
    n, C, HH, WW = feats.shape
    fpw = jnp.pad(feats, ((0, 0), (0, 0), (1, 1), (1, 1)))
    f3 = jnp.concatenate([fpw[:, :, :, dj:dj + WW] for dj in range(K)],
                         axis=1)  # (n, 39, H+2, W)
    out = None
    for di in range(K):
        v = f3[:, :, di:di + HH, :]
        # w1[:, c, di, dj] -> (o, 39) with dj-major blocks matching f3 order
        wdi = jnp.concatenate([w1[:, :, di, dj] for dj in range(K)], axis=1)
        p = jnp.einsum('nchw,oc->nohw', v, wdi)
        out = p if out is None else out + p
    return out  # (n,16,H,W)


def _conv2(feat, rw):
    n, C, HH, WW = feat.shape
    fp = jnp.pad(feat, ((0, 0), (0, 0), (1, 1), (1, 1)))
    out = None
    for di in range(K):
        for dj in range(K):
            v = fp[:, :, di:di + HH, dj:dj + WW]
            p = jnp.einsum('nchw,c->nhw', v, rw[0, :, di, dj])
            out = p if out is None else out + p
    return out[:, None]  # (n,1,H,W)


@partial(jax.pmap, in_axes=(0, None, None, None), devices=jax.devices()[:N_CORES])
def _run_shard(x, w1, rw, rb):
    y = _conv2(_conv1(_feats(x), w1), rw)
    return y + rb[None, :, None, None]


def _fold_weights(base_w, spline_w, spline_scaler):
    # scaled spline weights (16, 9, 8) -> relu-cube weights (16, 9, 12)
    sw = (np.asarray(spline_w) * np.asarray(spline_scaler)[..., None]).astype(np.float64)
    c5 = np.array([1.0, -4.0, 6.0, -4.0, 1.0], np.float64) / 6.0
    wr = np.zeros((N_CONVS, P, NR), np.float64)
    for g in range(G):
        for k in range(5):
            wr[:, :, g + k] += sw[:, :, g] * c5[k]
    bw = np.asarray(base_w, np.float64)
    w1 = np.zeros((N_CONVS, 1 + NR, K, K), np.float32)
    for di in range(K):
        for dj in range(K):
            p = di * K + dj
            w1[:, 0, di, dj] = bw[:, p]
            w1[:, 1:, di, dj] = wr[:, p, :]
    return w1


def kernel(x, base_w, spline_w, spline_scaler, restore_w, restore_b):
    x = np.asarray(x, np.float32)
    w1 = _fold_weights(base_w, spline_w, spline_scaler)
    xs = x.reshape(N_CORES, B // N_CORES, 1, H, W)
    y = _run_shard(xs, jnp.asarray(w1), jnp.asarray(restore_w, np.float32),
                   jnp.asarray(restore_b, np.float32))
    return np.asarray(y).reshape(B, 1, H, W)
